# revision 2
# baseline (speedup 1.0000x reference)
"""Bass/Trainium2 kernel for a 2-layer GAT (GATConv x2 + log_softmax) on 8 NeuronCores.

v3 design (edge/data parallel, dst-sharded, replicated node phase):
  - Shards padded to sh=ceil(N/NC/128)*128 nodes; core c owns dst nodes
    [c*sh, (c+1)*sh).  Table rows permuted (pi: half0 of every core first,
    then half1) so the l2tab AllGather can be emitted in 2 contiguous
    halves, the first overlapping the L1 edge-phase tail.
  - Phase A is REPLICATED: every core computes h|el for ALL nodes from the
    full (pi-ordered) feature matrix and writes its own htab copy -> no
    htab collective at all.  el/er come free out of the PE via host-extended
    weights [W1 | W1@al | W1@ar].  A tiny per-core pass over the core's own
    shard produces er (er_hilo, exact bf16 hi+lo pair, kept in SBUF).
  - Edge phase (dst-chunk blocked, software-pipelined preamble/body so
    in-order engines don't convoy): dma_gather h[src] 512B rows from htab;
    one-hot B[edge,slot] built on DVE; er[dst] per edge via PE transpose of
    B + matmul against er_hilo (no er gather, no didx); ex=exp(leaky(el+er));
    aggregate with PE matmuls psum += B^T @ [ex*h | ex]; normalize, bias,
    ELU -> fused L2 node phase (bf16 transpose + W2a matmul) -> l2shard.
  - L2 edge phase: same pipeline on 256B f32 rows; log_softmax two-pass
    (exp per chunk, single Ln at the end); one bulk output DMA.
"""

import os
import sys

import numpy as np

sys.path.insert(0, "/opt/trn_rl_repo")

import ml_dtypes

BF16 = ml_dtypes.bfloat16

# ---------------------------------------------------------------- constants
N_NODES = 100000
F_IN = 256
HID = 16
HEADS = 8
N_CLASSES = 16
NEG_SLOPE = 0.2
NC = 8                      # cores
CH = 128                    # dst nodes per chunk
BLK = 2                     # chunks per block (gather granularity)
GMAX = 8                    # max 128-edge tiles per dma_gather instruction
RING = 16384                # dynamic dma scratch bytes (1024-desc ring)
ROW1 = 256                  # bf16 per L1 table row (h 128 | el 16 | pad)
ROW2 = 64                   # f32 per L2 table row (hh 16 | el2 1 | pad)
WIN = 32768                 # rows per int16 gather window
ERG = 16                    # tiles per er-lookup psum group
TG = 4                      # tiles per transpose/copy group


def _wrap16(v):
    # [n] -> [128, n//16] int16; idx i at [i%16, i//16], replicated over groups
    n = v.shape[0]
    assert n % 16 == 0
    a = v.reshape(n // 16, 16).T.astype(np.int16)      # [16, n//16]
    return np.ascontiguousarray(np.tile(a, (8, 1)))    # [128, n//16]


def host_prep(src, dst, n_nodes=N_NODES, nc=NC, ch=CH):
    """Partition/sort/pad edges; build gather index + slot arrays per core."""
    sh = ((n_nodes + nc - 1) // nc + 127) // 128 * 128
    np_nodes = nc * sh
    ntile_a = sh // 128
    split_t = (ntile_a + 1) // 2
    h0 = split_t * 128
    h1 = sh - h0                         # rows in half 1 (may be 0)

    def pi(n):
        c, r = n // sh, n % sh
        return np.where(r < h0, c * h0 + r,
                        nc * h0 + c * max(h1, 1) + (r - h0))

    nchunk = sh // ch
    psrc = pi(src.astype(np.int64))
    nwin = (np_nodes + WIN - 1) // WIN
    bases = [w * WIN for w in range(nwin)]

    per_core = []
    cnt = np.zeros((nc, nchunk, nwin), dtype=np.int64)
    for c in range(nc):
        m = (dst // sh) == c
        es, ed = psrc[m], (dst[m] - c * sh).astype(np.int64)
        chunk = ed // ch
        bank = es // WIN
        order = np.lexsort((bank, chunk))
        es, ed, chunk, bank = es[order], ed[order], chunk[order], bank[order]
        per_core.append((es, ed))
        for k in range(nchunk):
            km = chunk == k
            for w in range(nwin):
                cnt[c, k, w] = np.count_nonzero(km & (bank == w))

    # uniform tile counts: max over cores, ceil to 128-edge tiles
    T = np.maximum(np.ceil(cnt.max(axis=0) / 128.0), 1).astype(np.int64)

    nblk = (nchunk + BLK - 1) // BLK
    # stream layout: for blk: for w: for k in blk: seg(k, w) of T[k,w] tiles
    seg_off = {}
    gath = []             # (block, window, tile_off, ntiles)
    toff = 0
    for b in range(nblk):
        ks = list(range(b * BLK, min((b + 1) * BLK, nchunk)))
        for w in range(nwin):
            g0 = toff
            for k in ks:
                seg_off[(k, w)] = toff
                toff += int(T[k, w])
            gath.append((b, w, g0, toff - g0))
    ntile = toff
    ne_pad = ntile * 128

    gidx = np.zeros((nc, ne_pad), dtype=np.int16)
    slot = np.full((nc, ne_pad), -1, dtype=np.float32)
    for c in range(nc):
        es, ed = per_core[c]
        p = 0
        ptr = {}
        for k in range(nchunk):
            for w in range(nwin):
                n = int(cnt[c, k, w])
                ptr[(k, w)] = (p, p + n)
                p += n
        for (k, w), off in seg_off.items():
            a, bnd = ptr[(k, w)]
            n = bnd - a
            pos = off * 128
            gidx[c, pos:pos + n] = (es[a:bnd] - bases[w]).astype(np.int16)
            slot[c, pos:pos + n] = (ed[a:bnd] % ch).astype(np.float32)

    chunk_tiles = []
    tile_chunk = np.zeros(ntile, dtype=np.int64)
    for k in range(nchunk):
        tl = []
        for w in range(nwin):
            off = seg_off[(k, w)]
            tl.extend(range(off, off + int(T[k, w])))
        chunk_tiles.append(tl)
        for t in tl:
            tile_chunk[t] = k

    # L2 AllGather split point: emit half-0 AG after the last block whose
    # chunks all lie in half 0 of the shard.
    split_chunk = h0 // ch
    l2_split_blk = split_chunk // BLK - 1 if split_chunk % BLK == 0 \
        else (split_chunk - 1) // BLK
    if l2_split_blk >= nblk - 1 or h1 == 0:
        l2_split_blk = None                # no useful split

    # table row -> node id (for building the pi-ordered feature matrix)
    j = np.arange(np_nodes, dtype=np.int64)
    row_node = np.where(
        j < nc * h0,
        (j // max(h0, 1)) * sh + (j % max(h0, 1)),
        ((j - nc * h0) // max(h1, 1)) * sh + h0 + (j - nc * h0) % max(h1, 1)
        if h1 > 0 else 0)

    return dict(
        n_nodes=n_nodes, np_nodes=np_nodes, sh=sh, nchunk=nchunk, nwin=nwin,
        bases=bases, nblk=nblk, ntile=ntile, ne_pad=ne_pad, gath=gath,
        chunk_tiles=chunk_tiles, tile_chunk=tile_chunk, seg_off=seg_off, T=T,
        gidx=gidx, slot=slot, h0=h0, h1=h1, split_t=split_t,
        l2_split_blk=l2_split_blk, row_node=row_node,
    )


# ------------------------------------------------------------- bass program
def build_program(meta, f_in, hid, heads, n_classes):
    from contextlib import ExitStack

    import concourse.tile as tile
    from concourse import bacc, mybir

    dt = mybir.dt
    f32, bf16, i16 = dt.float32, dt.bfloat16, dt.int16
    AF = mybir.ActivationFunctionType
    OP = mybir.AluOpType
    AX = mybir.AxisListType

    np_nodes = meta["np_nodes"]
    sh, nchunk, nwin = meta["sh"], meta["nchunk"], meta["nwin"]
    nblk, ntile, ne_pad = meta["nblk"], meta["ntile"], meta["ne_pad"]
    gath, chunk_tiles = meta["gath"], meta["chunk_tiles"]
    tile_chunk = meta["tile_chunk"]
    wbase = meta["bases"]
    h0, h1 = meta["h0"], meta["h1"]
    l2_split_blk = meta["l2_split_blk"]
    hd = heads * hid
    kt = f_in // 128
    we = hd + 2 * heads            # extended W1 columns: h | el | er
    ntile_a = sh // 128

    nc_ = bacc.Bacc("TRN2", target_bir_lowering=False, debug=False,
                    num_devices=NC, dynamic_dma_scratch_size=RING)

    def din(name, shape, dtype):
        return nc_.dram_tensor(name, list(shape), dtype,
                               kind="ExternalInput").ap()

    featT = din("featT", [128, kt, np_nodes], bf16)   # full, pi-ordered
    featO = din("featO", [128, kt, sh], bf16)         # own shard, natural
    W1e = din("W1e", [128, kt, we], bf16)
    b1rep = din("b1rep", [128, hd], f32)
    W2a = din("W2a", [hd, n_classes + 2], f32)
    b2rep = din("b2rep", [128, n_classes], f32)
    gidx_d = din("gidx", [128, ne_pad // 16], i16)
    slot_d = din("slot", [128, ntile], bf16)
    out_d = nc_.dram_tensor("out", [sh, n_classes], f32,
                            kind="ExternalOutput").ap()

    htab = nc_.dram_tensor("htab", [np_nodes, ROW1], bf16).ap()
    l2tab = nc_.dram_tensor("l2tab", [np_nodes, ROW2], f32).ap()
    l2shard = nc_.dram_tensor("l2shard", [sh, ROW2], f32).ap()

    replica = [list(range(NC))]

    with tile.TileContext(nc_) as tc:
        nc = tc.nc
        with ExitStack() as cctx:
            cpool = cctx.enter_context(tc.tile_pool(name="const", bufs=1))
            w1_sb = cpool.tile([128, kt * we], bf16, tag="w1")
            nc.sync.dma_start(
                w1_sb[:].rearrange("p (k o) -> p k o", k=kt), W1e[:])
            b1_sb = cpool.tile([128, hd], f32, tag="b1")
            nc.sync.dma_start(b1_sb[:], b1rep[:])
            w2_sb = cpool.tile([hd, n_classes + 2], f32, tag="w2")
            nc.sync.dma_start(w2_sb[:], W2a[:])
            b2_sb = cpool.tile([128, n_classes], f32, tag="b2")
            nc.sync.dma_start(b2_sb[:], b2rep[:])
            iota_sb = cpool.tile([128, 128], f32, tag="iota")
            nc.gpsimd.iota(iota_sb[:], pattern=[[1, 128]], base=0,
                           channel_multiplier=0,
                           allow_small_or_imprecise_dtypes=True)
            iota_p = cpool.tile([128, 1], f32, tag="iotap")
            nc.gpsimd.iota(iota_p[:], pattern=[[0, 1]], base=0,
                           channel_multiplier=1,
                           allow_small_or_imprecise_dtypes=True)
            ident_sb = cpool.tile([128, 128], f32, tag="ident")
            nc.vector.tensor_scalar(out=ident_sb[:], in0=iota_sb[:],
                                    scalar1=iota_p[:], scalar2=None,
                                    op0=OP.is_equal)
            iota_bf = cpool.tile([128, 128], bf16, tag="iotabf")
            nc.vector.tensor_copy(iota_bf[:], iota_sb[:])
            ident_bf = cpool.tile([128, 128], bf16, tag="identbf")
            nc.vector.tensor_copy(ident_bf[:], ident_sb[:])

            er_res = cpool.tile([128, nchunk, heads], f32, tag="erres")
            nc.vector.memset(er_res[:], 0.0)
            er2_res = cpool.tile([128, nchunk, 1], f32, tag="er2res")
            nc.vector.memset(er2_res[:], 0.0)
            xx_all = cpool.tile([128, nchunk, n_classes], f32, tag="xxall")
            ssum_all = cpool.tile([128, nchunk], f32, tag="ssall")
            nc.vector.memset(ssum_all[:], 1.0)
            lss_all = cpool.tile([128, nchunk], f32, tag="lssall")

            # ---------------- phase A0: own-shard er ----------------
            with ExitStack() as octx:
                opool = octx.enter_context(tc.tile_pool(name="phO", bufs=2))
                opsum = octx.enter_context(
                    tc.tile_pool(name="phOps", bufs=2, space="PSUM"))
                for t0_ in range(0, ntile_a, 4):
                    tn = min(4, ntile_a - t0_)
                    fo = opool.tile([128, kt, 4 * 128], bf16, tag="fo")
                    nc.sync.dma_start(
                        fo[:, :, :tn * 128],
                        featO[:, :, t0_ * 128:(t0_ + tn) * 128])
                    pso = opsum.tile([128, 4, 2 * heads], f32, tag="pso")
                    for j in range(tn):
                        for k in range(kt):
                            nc.tensor.matmul(
                                pso[:, j, :],
                                lhsT=fo[:, k, j * 128:(j + 1) * 128],
                                rhs=w1_sb[:, k * we + hd:(k + 1) * we],
                                start=(k == 0), stop=(k == kt - 1))
                    nc.vector.tensor_copy(
                        er_res[:, t0_:t0_ + tn, :],
                        pso[:, 0:tn, heads:2 * heads])

            # ---------------- phase A: replicated h|el sweep ----------------
            with ExitStack() as actx:
                apool = actx.enter_context(tc.tile_pool(name="phA", bufs=3))
                apsum = actx.enter_context(
                    tc.tile_pool(name="phAps", bufs=4, space="PSUM"))
                for t0_ in range(0, np_nodes // 128, 4):
                    ft = apool.tile([128, kt, 4 * 128], bf16, tag="ft")
                    nc.sync.dma_start(
                        ft[:, :, :], featT[:, :, t0_ * 128:(t0_ + 4) * 128])
                    row = apool.tile([128, 4, ROW1], bf16, tag="row")
                    for j in range(4):
                        ps = apsum.tile([128, we], f32, tag="hps")
                        for k in range(kt):
                            nc.tensor.matmul(
                                ps[:, :],
                                lhsT=ft[:, k, j * 128:(j + 1) * 128],
                                rhs=w1_sb[:, k * we:(k + 1) * we],
                                start=(k == 0), stop=(k == kt - 1))
                        nc.scalar.copy(row[:, j, 0:hd], ps[:, 0:hd])
                        nc.vector.tensor_copy(
                            row[:, j, hd:hd + 2 * heads].bitcast(f32),
                            ps[:, hd:hd + heads])
                    nc.vector.memset(row[:, :, hd + 2 * heads:ROW1], 0.0)
                    nc.sync.dma_start(
                        htab[t0_ * 128:(t0_ + 4) * 128, :].rearrange(
                            "(j p) r -> p j r", p=128),
                        row[:])

            # ---------------- edge phases ----------------
            def post_chunk_l1(k, ps, ppost, ppsT, pps2):
                fw, sw = hd, heads
                den = ppost.tile([128, sw], f32, tag="den")
                nc.vector.tensor_scalar_max(den[:], ps[:, fw:fw + sw], 1e-30)
                rec = ppost.tile([128, sw], f32, tag="rec")
                nc.vector.reciprocal(rec[:], den[:])
                h2 = ppost.tile([128, fw], f32, tag="h2")
                nc.vector.tensor_mul(
                    h2[:].rearrange("p (s d) -> p s d", s=sw),
                    ps[:, 0:fw].rearrange("p (s d) -> p s d", s=sw),
                    rec[:].unsqueeze(2).broadcast_to([128, sw, fw // sw]))
                nc.vector.tensor_add(h2[:], h2[:], b1_sb[:])
                mn = ppost.tile([128, fw], f32, tag="mn")
                nc.vector.tensor_scalar_min(mn[:], h2[:], 0.0)
                nc.scalar.activation(mn[:], mn[:], AF.Exp)
                nc.vector.scalar_tensor_tensor(
                    out=h2[:], in0=h2[:], scalar=0.0,
                    in1=mn[:], op0=OP.max, op1=OP.add)
                nc.vector.tensor_scalar_sub(h2[:], h2[:], 1.0)
                # L2 node phase (f32 transpose + W2 matmul)
                pst = ppsT.tile([128, TG, 128], f32, tag="btp")
                nc.tensor.transpose(pst[:, 0, :], h2[:], ident_sb[:])
                h2T = ppost.tile([128, 128], f32, tag="h2T")
                nc.scalar.copy(h2T[:], pst[:, 0, :])
                ps2 = pps2.tile([128, n_classes + 2], f32, tag="hh")
                nc.tensor.matmul(ps2[:], lhsT=h2T[:], rhs=w2_sb[:],
                                 start=True, stop=True)
                l2r = ppost.tile([128, ROW2], f32, tag="l2r")
                nc.scalar.copy(l2r[:, 0:n_classes + 1],
                               ps2[:, 0:n_classes + 1])
                nc.vector.memset(l2r[:, n_classes + 1:ROW2], 0.0)
                nc.vector.tensor_copy(er2_res[:, k, :],
                                      ps2[:, n_classes + 1:n_classes + 2])
                nc.sync.dma_start(l2shard[k * CH:(k + 1) * CH, :], l2r[:])

            def post_chunk_l2(k, ps, ppost):
                fw = n_classes
                den = ppost.tile([128, 1], f32, tag="den2")
                nc.vector.tensor_scalar_max(den[:], ps[:, fw:fw + 1], 1e-30)
                rec = ppost.tile([128, 1], f32, tag="rec2")
                nc.vector.reciprocal(rec[:], den[:])
                xx = ppost.tile([128, fw], f32, tag="xx")
                nc.vector.tensor_scalar(out=xx[:], in0=ps[:, 0:fw],
                                        scalar1=rec[:], scalar2=None,
                                        op0=OP.mult)
                nc.vector.tensor_add(xx[:], xx[:], b2_sb[:])
                rmax = ppost.tile([128, 1], f32, tag="rmax")
                nc.vector.tensor_reduce(out=rmax[:], in_=xx[:],
                                        axis=AX.X, op=OP.max)
                nc.vector.tensor_scalar(out=xx_all[:, k, :], in0=xx[:],
                                        scalar1=rmax[:], scalar2=None,
                                        op0=OP.subtract)
                exs = ppost.tile([128, fw], f32, tag="exs")
                nc.scalar.activation(exs[:], xx_all[:, k, :], AF.Exp,
                                     accum_out=ssum_all[:, k:k + 1])

            def edge_phase(layer):
                if layer == 1:
                    tab, rw, fw, sw = htab, ROW1, hd, heads
                    gdt, erhl = bf16, er_res
                else:
                    tab, rw, fw, sw = l2tab, ROW2, n_classes, 1
                    gdt, erhl = f32, er2_res
                nw = fw + sw

                gblocks = {}
                for (b, w, g0, nt) in gath:
                    gblocks.setdefault(b, []).append((w, g0, nt))

                with ExitStack() as ectx:
                    pool = ectx.enter_context(
                        tc.tile_pool(name=f"edge{layer}", bufs=2))
                    pps = ectx.enter_context(
                        tc.tile_pool(name=f"eps{layer}", bufs=2,
                                     space="PSUM"))
                    ppsT = ectx.enter_context(
                        tc.tile_pool(name=f"epsT{layer}", bufs=2,
                                     space="PSUM"))
                    pper = ectx.enter_context(
                        tc.tile_pool(name=f"epsE{layer}", bufs=2,
                                     space="PSUM"))
                    ppost = ectx.enter_context(
                        tc.tile_pool(name=f"post{layer}", bufs=2))
                    pps2 = ectx.enter_context(
                        tc.tile_pool(name=f"ep2{layer}", bufs=2,
                                     space="PSUM")) if layer == 1 else None

                    def preamble(b):
                        segs = gblocks[b]
                        t0 = segs[0][1]
                        tb = sum(s[2] for s in segs)
                        gi = pool.tile([128, tb * 8], i16, tag="gi")
                        nc.sync.dma_start(gi[:],
                                          gidx_d[:, t0 * 8:(t0 + tb) * 8])
                        sl = pool.tile([128, tb], bf16, tag="sl")
                        nc.sync.dma_start(sl[:], slot_d[:, t0:t0 + tb])
                        gt = pool.tile([128, tb, rw], gdt, tag="gt")
                        for (w, g0, nt) in segs:
                            if nt == 0:
                                continue
                            wend = min(wbase[w] + WIN, np_nodes)
                            for s0 in range(0, nt, GMAX):
                                sn = min(GMAX, nt - s0)
                                off = g0 - t0 + s0
                                nc.gpsimd.dma_gather(
                                    out_ap=gt[:, off:off + sn, :],
                                    in_ap=tab[wbase[w]:wend, :],
                                    idxs_ap=gi[:, off * 8:(off + sn) * 8],
                                    num_idxs=sn * 128,
                                    num_idxs_reg=sn * 128, elem_size=rw,
                                    queue_num=0)
                        B = pool.tile([128, tb, 128], bf16, tag="B")
                        nc.vector.tensor_tensor(
                            out=B[:],
                            in0=iota_bf[:].unsqueeze(1)
                            .broadcast_to([128, tb, 128]),
                            in1=sl[:].unsqueeze(2)
                            .broadcast_to([128, tb, 128]),
                            op=OP.is_equal)
                        exw = pool.tile([128, tb, sw], f32, tag="exw")
                        Bf = pool.tile([128, tb, 128], f32, tag="Bf")
                        nc.vector.tensor_copy(Bf[:], B[:])
                        for q0 in range(0, tb, ERG):
                            qn = min(ERG, tb - q0)
                            erp = pper.tile([128, ERG, sw], f32,
                                            tag="erps")
                            for c0 in range(q0, q0 + qn, TG):
                                cn = min(TG, q0 + qn - c0)
                                btp = ppsT.tile([128, TG, 128], f32,
                                                tag="btp")
                                for i in range(cn):
                                    nc.tensor.transpose(
                                        btp[:, i, :], Bf[:, c0 + i, :],
                                        ident_sb[:])
                                bts = pool.tile([128, TG, 128], f32,
                                                tag="bts")
                                nc.scalar.copy(bts[:, 0:cn, :],
                                               btp[:, 0:cn, :])
                                for i in range(cn):
                                    k = int(tile_chunk[t0 + c0 + i])
                                    nc.tensor.matmul(
                                        erp[:, c0 + i - q0, :],
                                        lhsT=bts[:, i, :],
                                        rhs=erhl[:, k, :],
                                        start=True, stop=True)
                            nc.vector.tensor_copy(exw[:, q0:q0 + qn, :],
                                                  erp[:, 0:qn, :])
                        return dict(b=b, t0=t0, tb=tb, gt=gt, B=B, exw=exw)

                    def body(st):
                        b, t0, tb = st["b"], st["t0"], st["tb"]
                        gt, B, exw = st["gt"], st["B"], st["exw"]
                        if layer == 1:
                            el_ap = gt[:, :, fw:fw + 2 * sw].bitcast(f32)
                        else:
                            el_ap = gt[:, :, fw:fw + sw]
                        nc.vector.tensor_add(exw[:], exw[:], el_ap)
                        nc.vector.scalar_tensor_tensor(
                            out=exw[:], in0=exw[:], scalar=NEG_SLOPE,
                            in1=exw[:], op0=OP.mult, op1=OP.max)
                        comb = pool.tile([128, tb, nw], bf16, tag="comb")
                        nc.scalar.activation(comb[:, :, fw:fw + sw], exw[:],
                                             AF.Exp)
                        nc.vector.tensor_mul(
                            comb[:, :, 0:fw].rearrange(
                                "p t (s d) -> p t s d", s=sw),
                            gt[:, :, 0:fw].rearrange(
                                "p t (s d) -> p t s d", s=sw),
                            comb[:, :, fw:fw + sw].unsqueeze(3)
                            .broadcast_to([128, tb, sw, fw // sw]))
                        for k in range(b * BLK, min((b + 1) * BLK, nchunk)):
                            tl = chunk_tiles[k]
                            ps = pps.tile([128, nw], f32, tag="agg")
                            for j, t in enumerate(tl):
                                nc.tensor.matmul(
                                    ps[:], lhsT=B[:, t - t0, :],
                                    rhs=comb[:, t - t0, :],
                                    start=(j == 0), stop=(j == len(tl) - 1))
                            if layer == 1:
                                post_chunk_l1(k, ps, ppost, ppsT, pps2)
                            else:
                                post_chunk_l2(k, ps, ppost)
                        if layer == 1 and l2_split_blk is not None \
                                and b == l2_split_blk:
                            nc.gpsimd.collective_compute(
                                "AllGather", OP.bypass,
                                replica_groups=replica,
                                ins=[l2shard[0:h0, :]],
                                outs=[l2tab[0:NC * h0, :]])

                    st = preamble(0)
                    for b in range(1, nblk):
                        st2 = preamble(b)
                        body(st)
                        st = st2
                    body(st)

            edge_phase(1)
            if l2_split_blk is not None:
                nc.gpsimd.collective_compute(
                    "AllGather", OP.bypass, replica_groups=replica,
                    ins=[l2shard[h0:sh, :]],
                    outs=[l2tab[NC * h0:np_nodes, :]])
            else:
                nc.gpsimd.collective_compute(
                    "AllGather", OP.bypass, replica_groups=replica,
                    ins=[l2shard[:]], outs=[l2tab[:]])
            edge_phase(2)

            # two-pass log-softmax epilogue
            nc.scalar.activation(lss_all[:], ssum_all[:], AF.Ln)
            nc.vector.tensor_tensor(
                out=xx_all[:], in0=xx_all[:],
                in1=lss_all[:].unsqueeze(2)
                .broadcast_to([128, nchunk, n_classes]),
                op=OP.subtract)
            nc.sync.dma_start(
                out_d[:].rearrange("(k p) c -> p k c", p=128),
                xx_all[:])

    nc_.compile()
    return nc_


# ------------------------------------------------------------------ driver
def make_in_maps(meta, feat, W1, al1, ar1, b1, W2, al2, ar2, b2):
    sh, np_nodes = meta["sh"], meta["np_nodes"]
    n_real, f_in = feat.shape
    kt = f_in // 128
    hd = W1.shape[1]
    heads, hidd = al1.shape if al1.ndim == 2 else (1, al1.shape[0])
    W1al = np.einsum("fhd,hd->fh", W1.reshape(f_in, heads, hidd), al1)
    W1ar = np.einsum("fhd,hd->fh", W1.reshape(f_in, heads, hidd), ar1)
    W1e = np.concatenate([W1, W1al, W1ar], axis=1)          # [f_in, we]
    we = W1e.shape[1]
    W1er = np.ascontiguousarray(
        W1e.reshape(kt, 128, we).transpose(1, 0, 2)).astype(BF16)
    b1rep = np.tile(b1.reshape(1, -1), (128, 1)).astype(np.float32)
    W2a = np.concatenate(
        [W2, W2 @ al2.reshape(-1, 1), W2 @ ar2.reshape(-1, 1)],
        axis=1).astype(np.float32)
    b2rep = np.tile(b2.reshape(1, -1), (128, 1)).astype(np.float32)

    featP = np.zeros((np_nodes, f_in), dtype=np.float32)
    featP[:n_real] = feat
    fpi = featP[meta["row_node"]]                            # pi-ordered
    featT = np.ascontiguousarray(
        fpi.T.reshape(kt, 128, np_nodes).transpose(1, 0, 2)).astype(BF16)

    in_maps = []
    for c in range(NC):
        fo = featP[c * sh:(c + 1) * sh]
        featO = np.ascontiguousarray(
            fo.T.reshape(kt, 128, sh).transpose(1, 0, 2)).astype(BF16)
        in_maps.append({
            "featT": featT, "featO": featO,
            "W1e": W1er, "b1rep": b1rep,
            "W2a": W2a, "b2rep": b2rep,
            "gidx": _wrap16(meta["gidx"][c]),
            "slot": np.ascontiguousarray(
                meta["slot"][c].reshape(-1, 128).T).astype(BF16),
        })
    return in_maps


class Runner:
    """Builds the SPMD program once; exposes a repeatable timed executor."""

    def __init__(self, meta, f_in):
        self.meta = meta
        self.nc = build_program(meta, f_in, HID, HEADS, N_CLASSES)
        self._fn = None
        self.repeat = 1

    repeat = 1

    def _lower(self):
        import jax
        import numpy as _np
        from jax.sharding import Mesh, PartitionSpec
        from jax.experimental.shard_map import shard_map
        from concourse import mybir
        from concourse.bass2jax import _bass_exec_p, install_neuronx_cc_hook

        install_neuronx_cc_hook()
        nc = self.nc
        in_names, out_names, out_avals, zero_outs = [], [], [], []
        partition_name = (nc.partition_id_tensor.name
                          if nc.partition_id_tensor else None)
        for alloc in nc.m.functions[0].allocations:
            if not isinstance(alloc, mybir.MemoryLocationSet):
                continue
            name = alloc.memorylocations[0].name
            if alloc.kind == "ExternalInput":
                if name != partition_name:
                    in_names.append(name)
            elif alloc.kind == "ExternalOutput":
                shape = tuple(alloc.tensor_shape)
                dtype = mybir.dt.np(alloc.dtype)
                out_names.append(name)
                out_avals.append(jax.core.ShapedArray(shape, dtype))
                zero_outs.append(_np.zeros(shape, dtype))
        n_params = len(in_names)
        n_outs = len(out_avals)
        all_in_names = list(in_names) + list(out_names)
        if partition_name is not None:
            all_in_names.append(partition_name)

        repeat = self.repeat

        def _body(*args):
            ins = list(args[:n_params])
            zouts = list(args[n_params:])
            outs = None
            for _ in range(repeat):
                operands = ins + zouts
                if partition_name is not None:
                    from concourse.bass2jax import partition_id_tensor
                    operands.append(partition_id_tensor())
                outs = _bass_exec_p.bind(
                    *operands, out_avals=tuple(out_avals),
                    in_names=tuple(all_in_names), out_names=tuple(out_names),
                    lowering_input_output_aliases=(),
                    sim_require_finite=False, sim_require_nnan=False, nc=nc)
                zouts = list(outs)   # chain: serialize + reuse as out bufs
            return tuple(outs)

        devices = jax.devices()[:NC]
        mesh = Mesh(_np.asarray(devices), ("core",))
        in_specs = (PartitionSpec("core"),) * (n_params + n_outs)
        out_specs = (PartitionSpec("core"),) * n_outs
        self._fn = jax.jit(
            shard_map(_body, mesh=mesh, in_specs=in_specs,
                      out_specs=out_specs, check_rep=False),
            keep_unused=True)
        self._in_names = in_names
        self._out_names = out_names
        self._out_avals = out_avals
        self._zero_outs = zero_outs
        self._mesh = mesh
        self._in_specs = in_specs

    def prepare(self, in_maps):
        import jax
        import numpy as _np
        from jax.sharding import NamedSharding, PartitionSpec
        if self._fn is None:
            self._lower()
        concat_in = [
            _np.concatenate([in_maps[c][name] for c in range(NC)], axis=0)
            for name in self._in_names]
        concat_zeros = [
            _np.zeros((NC * z.shape[0], *z.shape[1:]), z.dtype)
            for z in self._zero_outs]
        shd = NamedSharding(self._mesh, PartitionSpec("core"))
        self._args = [jax.device_put(a, shd) for a in concat_in + concat_zeros]
        jax.block_until_ready(self._args)

    def run(self):
        import jax
        out = self._fn(*self._args)
        out = jax.block_until_ready(out)
        import numpy as _np
        res = _np.asarray(out[self._out_names.index("out")])
        sh = self._out_avals[self._out_names.index("out")].shape
        return res.reshape(NC, *sh).reshape(NC * sh[0], *sh[1:])


_RUNNER = None


def get_runner(feat, src, dst):
    global _RUNNER
    n, f_in = feat.shape
    meta = host_prep(np.asarray(src, np.int32), np.asarray(dst, np.int32),
                     n_nodes=n)
    _RUNNER = Runner(meta, f_in)
    return _RUNNER


def kernel(feat, src, dst, W1, al1, ar1, b1, W2, al2, ar2, b2):
    feat = np.asarray(feat, dtype=np.float32)
    src = np.asarray(src, dtype=np.int32)
    dst = np.asarray(dst, dtype=np.int32)
    args = [np.asarray(x, np.float32)
            for x in (W1, al1, ar1, b1, W2, al2, ar2, b2)]
    r = _RUNNER if _RUNNER is not None else get_runner(feat, src, dst)
    in_maps = make_in_maps(r.meta, feat, *args)
    r.prepare(in_maps)
    return r.run()[:feat.shape[0]]


kernel.last_exec_time_ns = None


# revision 3
# speedup vs baseline: 1.0209x; 1.0209x over previous
"""Bass/Trainium2 kernel for a 2-layer GAT (GATConv x2 + log_softmax) on 8 NeuronCores.

v3 design (edge/data parallel, dst-sharded, replicated node phase):
  - Shards padded to sh=ceil(N/NC/128)*128 nodes; core c owns dst nodes
    [c*sh, (c+1)*sh).  Table rows permuted (pi: half0 of every core first,
    then half1) so the l2tab AllGather can be emitted in 2 contiguous
    halves, the first overlapping the L1 edge-phase tail.
  - Phase A is REPLICATED: every core computes h|el for ALL nodes from the
    full (pi-ordered) feature matrix and writes its own htab copy -> no
    htab collective at all.  el/er come free out of the PE via host-extended
    weights [W1 | W1@al | W1@ar].  A tiny per-core pass over the core's own
    shard produces er (er_hilo, exact bf16 hi+lo pair, kept in SBUF).
  - Edge phase (dst-chunk blocked, software-pipelined preamble/body so
    in-order engines don't convoy): dma_gather h[src] 512B rows from htab;
    one-hot B[edge,slot] built on DVE; er[dst] per edge via PE transpose of
    B + matmul against er_hilo (no er gather, no didx); ex=exp(leaky(el+er));
    aggregate with PE matmuls psum += B^T @ [ex*h | ex]; normalize, bias,
    ELU -> fused L2 node phase (bf16 transpose + W2a matmul) -> l2shard.
  - L2 edge phase: same pipeline on 256B f32 rows; log_softmax two-pass
    (exp per chunk, single Ln at the end); one bulk output DMA.
"""

import os
import sys

import numpy as np

sys.path.insert(0, "/opt/trn_rl_repo")

import ml_dtypes

BF16 = ml_dtypes.bfloat16

# ---------------------------------------------------------------- constants
N_NODES = 100000
F_IN = 256
HID = 16
HEADS = 8
N_CLASSES = 16
NEG_SLOPE = 0.2
NC = 8                      # cores
CH = 128                    # dst nodes per chunk
BLK = 2                     # chunks per block (gather granularity)
GMAX = 8                    # max 128-edge tiles per dma_gather instruction
RING = 16384                # dynamic dma scratch bytes (1024-desc ring)
ROW1 = 256                  # bf16 per L1 table row (h 128 | el 16 | pad)
ROW2 = 64                   # f32 per L2 table row (hh 16 | el2 1 | pad)
WIN = 32768                 # rows per int16 gather window
ERG = 16                    # tiles per er-lookup psum group
TG = 4                      # tiles per transpose/copy group


def _wrap16(v):
    # [n] -> [128, n//16] int16; idx i at [i%16, i//16], replicated over groups
    n = v.shape[0]
    assert n % 16 == 0
    a = v.reshape(n // 16, 16).T.astype(np.int16)      # [16, n//16]
    return np.ascontiguousarray(np.tile(a, (8, 1)))    # [128, n//16]


def host_prep(src, dst, n_nodes=N_NODES, nc=NC, ch=CH):
    """Partition/sort/pad edges; build gather index + slot arrays per core."""
    sh = ((n_nodes + nc - 1) // nc + 127) // 128 * 128
    np_nodes = nc * sh
    ntile_a = sh // 128
    split_t = (ntile_a + 1) // 2
    h0 = split_t * 128
    h1 = sh - h0                         # rows in half 1 (may be 0)

    def pi(n):
        c, r = n // sh, n % sh
        return np.where(r < h0, c * h0 + r,
                        nc * h0 + c * max(h1, 1) + (r - h0))

    nchunk = sh // ch
    psrc = pi(src.astype(np.int64))
    nwin = (np_nodes + WIN - 1) // WIN
    bases = [w * WIN for w in range(nwin)]

    per_core = []
    cnt = np.zeros((nc, nchunk, nwin), dtype=np.int64)
    for c in range(nc):
        m = (dst // sh) == c
        es, ed = psrc[m], (dst[m] - c * sh).astype(np.int64)
        chunk = ed // ch
        bank = es // WIN
        order = np.lexsort((bank, chunk))
        es, ed, chunk, bank = es[order], ed[order], chunk[order], bank[order]
        per_core.append((es, ed))
        for k in range(nchunk):
            km = chunk == k
            for w in range(nwin):
                cnt[c, k, w] = np.count_nonzero(km & (bank == w))

    # uniform tile counts: max over cores, ceil to 128-edge tiles
    T = np.maximum(np.ceil(cnt.max(axis=0) / 128.0), 1).astype(np.int64)

    nblk = (nchunk + BLK - 1) // BLK
    # stream layout: for blk: for w: for k in blk: seg(k, w) of T[k,w] tiles
    seg_off = {}
    gath = []             # (block, window, tile_off, ntiles)
    toff = 0
    for b in range(nblk):
        ks = list(range(b * BLK, min((b + 1) * BLK, nchunk)))
        for w in range(nwin):
            g0 = toff
            for k in ks:
                seg_off[(k, w)] = toff
                toff += int(T[k, w])
            gath.append((b, w, g0, toff - g0))
    ntile = toff
    ne_pad = ntile * 128

    gidx = np.zeros((nc, ne_pad), dtype=np.int16)
    slot = np.full((nc, ne_pad), -1, dtype=np.float32)
    for c in range(nc):
        es, ed = per_core[c]
        p = 0
        ptr = {}
        for k in range(nchunk):
            for w in range(nwin):
                n = int(cnt[c, k, w])
                ptr[(k, w)] = (p, p + n)
                p += n
        for (k, w), off in seg_off.items():
            a, bnd = ptr[(k, w)]
            n = bnd - a
            pos = off * 128
            gidx[c, pos:pos + n] = (es[a:bnd] - bases[w]).astype(np.int16)
            slot[c, pos:pos + n] = (ed[a:bnd] % ch).astype(np.float32)

    chunk_tiles = []
    tile_chunk = np.zeros(ntile, dtype=np.int64)
    for k in range(nchunk):
        tl = []
        for w in range(nwin):
            off = seg_off[(k, w)]
            tl.extend(range(off, off + int(T[k, w])))
        chunk_tiles.append(tl)
        for t in tl:
            tile_chunk[t] = k

    # L2 AllGather split point: emit half-0 AG after the last block whose
    # chunks all lie in half 0 of the shard.
    split_chunk = h0 // ch
    l2_split_blk = split_chunk // BLK - 1 if split_chunk % BLK == 0 \
        else (split_chunk - 1) // BLK
    if l2_split_blk >= nblk - 1 or h1 == 0:
        l2_split_blk = None                # no useful split

    # table row -> node id (for building the pi-ordered feature matrix)
    j = np.arange(np_nodes, dtype=np.int64)
    row_node = np.where(
        j < nc * h0,
        (j // max(h0, 1)) * sh + (j % max(h0, 1)),
        ((j - nc * h0) // max(h1, 1)) * sh + h0 + (j - nc * h0) % max(h1, 1)
        if h1 > 0 else 0)

    return dict(
        n_nodes=n_nodes, np_nodes=np_nodes, sh=sh, nchunk=nchunk, nwin=nwin,
        bases=bases, nblk=nblk, ntile=ntile, ne_pad=ne_pad, gath=gath,
        chunk_tiles=chunk_tiles, tile_chunk=tile_chunk, seg_off=seg_off, T=T,
        gidx=gidx, slot=slot, h0=h0, h1=h1, split_t=split_t,
        l2_split_blk=l2_split_blk, row_node=row_node,
    )


# ------------------------------------------------------------- bass program
def build_program(meta, f_in, hid, heads, n_classes):
    from contextlib import ExitStack

    import concourse.tile as tile
    from concourse import bacc, mybir

    dt = mybir.dt
    f32, bf16, i16 = dt.float32, dt.bfloat16, dt.int16
    AF = mybir.ActivationFunctionType
    OP = mybir.AluOpType
    AX = mybir.AxisListType

    np_nodes = meta["np_nodes"]
    sh, nchunk, nwin = meta["sh"], meta["nchunk"], meta["nwin"]
    nblk, ntile, ne_pad = meta["nblk"], meta["ntile"], meta["ne_pad"]
    gath, chunk_tiles = meta["gath"], meta["chunk_tiles"]
    tile_chunk = meta["tile_chunk"]
    wbase = meta["bases"]
    h0, h1 = meta["h0"], meta["h1"]
    l2_split_blk = meta["l2_split_blk"]
    hd = heads * hid
    kt = f_in // 128
    we = hd + 2 * heads            # extended W1 columns: h | el | er
    ntile_a = sh // 128

    nc_ = bacc.Bacc("TRN2", target_bir_lowering=False, debug=False,
                    num_devices=NC, dynamic_dma_scratch_size=RING)

    def din(name, shape, dtype):
        return nc_.dram_tensor(name, list(shape), dtype,
                               kind="ExternalInput").ap()

    featT = din("featT", [128, kt, np_nodes], bf16)   # full, pi-ordered
    featO = din("featO", [128, kt, sh], bf16)         # own shard, natural
    W1e = din("W1e", [128, kt, we], bf16)
    b1rep = din("b1rep", [128, hd], f32)
    W2a = din("W2a", [hd, n_classes + 2], bf16)
    b2rep = din("b2rep", [128, n_classes], f32)
    gidx_d = din("gidx", [128, ne_pad // 16], i16)
    slot_d = din("slot", [128, ntile], bf16)
    out_d = nc_.dram_tensor("out", [sh, n_classes], f32,
                            kind="ExternalOutput").ap()

    htab = nc_.dram_tensor("htab", [np_nodes, ROW1], bf16).ap()
    l2tab = nc_.dram_tensor("l2tab", [np_nodes, ROW2], f32).ap()
    l2shard = nc_.dram_tensor("l2shard", [sh, ROW2], f32).ap()

    replica = [list(range(NC))]

    with tile.TileContext(nc_) as tc:
        nc = tc.nc
        with ExitStack() as cctx:
            cpool = cctx.enter_context(tc.tile_pool(name="const", bufs=1))
            w1_sb = cpool.tile([128, kt * we], bf16, tag="w1")
            nc.sync.dma_start(
                w1_sb[:].rearrange("p (k o) -> p k o", k=kt), W1e[:])
            b1_sb = cpool.tile([128, hd], f32, tag="b1")
            nc.sync.dma_start(b1_sb[:], b1rep[:])
            w2_sb = cpool.tile([hd, n_classes + 2], bf16, tag="w2")
            nc.sync.dma_start(w2_sb[:], W2a[:])
            b2_sb = cpool.tile([128, n_classes], f32, tag="b2")
            nc.sync.dma_start(b2_sb[:], b2rep[:])
            iota_sb = cpool.tile([128, 128], f32, tag="iota")
            nc.gpsimd.iota(iota_sb[:], pattern=[[1, 128]], base=0,
                           channel_multiplier=0,
                           allow_small_or_imprecise_dtypes=True)
            iota_p = cpool.tile([128, 1], f32, tag="iotap")
            nc.gpsimd.iota(iota_p[:], pattern=[[0, 1]], base=0,
                           channel_multiplier=1,
                           allow_small_or_imprecise_dtypes=True)
            ident_sb = cpool.tile([128, 128], f32, tag="ident")
            nc.vector.tensor_scalar(out=ident_sb[:], in0=iota_sb[:],
                                    scalar1=iota_p[:], scalar2=None,
                                    op0=OP.is_equal)
            iota_bf = cpool.tile([128, 128], bf16, tag="iotabf")
            nc.vector.tensor_copy(iota_bf[:], iota_sb[:])
            ident_bf = cpool.tile([128, 128], bf16, tag="identbf")
            nc.vector.tensor_copy(ident_bf[:], ident_sb[:])

            er_hilo = cpool.tile([128, nchunk, 2 * heads], bf16, tag="erhl")
            nc.vector.memset(er_hilo[:], 0.0)
            er2_hilo = cpool.tile([128, nchunk, 2], bf16, tag="er2hl")
            nc.vector.memset(er2_hilo[:], 0.0)
            xx_all = cpool.tile([128, nchunk, n_classes], f32, tag="xxall")
            ssum_all = cpool.tile([128, nchunk], f32, tag="ssall")
            nc.vector.memset(ssum_all[:], 1.0)
            lss_all = cpool.tile([128, nchunk], f32, tag="lssall")

            # ---------------- phase A0: own-shard er ----------------
            with ExitStack() as octx:
                opool = octx.enter_context(tc.tile_pool(name="phO", bufs=2))
                opsum = octx.enter_context(
                    tc.tile_pool(name="phOps", bufs=2, space="PSUM"))
                for t0_ in range(0, ntile_a, 4):
                    tn = min(4, ntile_a - t0_)
                    fo = opool.tile([128, kt, 4 * 128], bf16, tag="fo")
                    nc.sync.dma_start(
                        fo[:, :, :tn * 128],
                        featO[:, :, t0_ * 128:(t0_ + tn) * 128])
                    pso = opsum.tile([128, 4, 2 * heads], f32, tag="pso")
                    for j in range(tn):
                        for k in range(kt):
                            nc.tensor.matmul(
                                pso[:, j, :],
                                lhsT=fo[:, k, j * 128:(j + 1) * 128],
                                rhs=w1_sb[:, k * we + hd:(k + 1) * we],
                                start=(k == 0), stop=(k == kt - 1))
                    # er hi/lo (exact bf16 pair); er lives in cols heads:2*heads
                    nc.vector.tensor_copy(
                        er_hilo[:, t0_:t0_ + tn, 0:heads],
                        pso[:, 0:tn, heads:2 * heads])
                    hi_f = opool.tile([128, 4, heads], f32, tag="hif")
                    nc.vector.tensor_copy(
                        hi_f[:, 0:tn, :], er_hilo[:, t0_:t0_ + tn, 0:heads])
                    nc.vector.tensor_tensor(
                        out=er_hilo[:, t0_:t0_ + tn, heads:2 * heads],
                        in0=pso[:, 0:tn, heads:2 * heads],
                        in1=hi_f[:, 0:tn, :],
                        op=OP.subtract)

            # ---------------- phase A: replicated h|el sweep ----------------
            with ExitStack() as actx:
                apool = actx.enter_context(tc.tile_pool(name="phA", bufs=3))
                apsum = actx.enter_context(
                    tc.tile_pool(name="phAps", bufs=4, space="PSUM"))
                for t0_ in range(0, np_nodes // 128, 4):
                    ft = apool.tile([128, kt, 4 * 128], bf16, tag="ft")
                    nc.sync.dma_start(
                        ft[:, :, :], featT[:, :, t0_ * 128:(t0_ + 4) * 128])
                    row = apool.tile([128, 4, ROW1], bf16, tag="row")
                    for j in range(4):
                        ps = apsum.tile([128, we], f32, tag="hps")
                        for k in range(kt):
                            nc.tensor.matmul(
                                ps[:, :],
                                lhsT=ft[:, k, j * 128:(j + 1) * 128],
                                rhs=w1_sb[:, k * we:(k + 1) * we],
                                start=(k == 0), stop=(k == kt - 1))
                        nc.scalar.copy(row[:, j, 0:hd], ps[:, 0:hd])
                        nc.vector.tensor_copy(
                            row[:, j, hd:hd + 2 * heads].bitcast(f32),
                            ps[:, hd:hd + heads])
                    nc.vector.memset(row[:, :, hd + 2 * heads:ROW1], 0.0)
                    nc.sync.dma_start(
                        htab[t0_ * 128:(t0_ + 4) * 128, :].rearrange(
                            "(j p) r -> p j r", p=128),
                        row[:])

            # ---------------- edge phases ----------------
            def post_chunk_l1(k, ps, ppost, ppsT, pps2):
                fw, sw = hd, heads
                den = ppost.tile([128, sw], f32, tag="den")
                nc.vector.tensor_scalar_max(den[:], ps[:, fw:fw + sw], 1e-30)
                rec = ppost.tile([128, sw], f32, tag="rec")
                nc.vector.reciprocal(rec[:], den[:])
                h2 = ppost.tile([128, fw], f32, tag="h2")
                nc.vector.tensor_mul(
                    h2[:].rearrange("p (s d) -> p s d", s=sw),
                    ps[:, 0:fw].rearrange("p (s d) -> p s d", s=sw),
                    rec[:].unsqueeze(2).broadcast_to([128, sw, fw // sw]))
                nc.vector.tensor_add(h2[:], h2[:], b1_sb[:])
                mn = ppost.tile([128, fw], f32, tag="mn")
                nc.vector.tensor_scalar_min(mn[:], h2[:], 0.0)
                nc.scalar.activation(mn[:], mn[:], AF.Exp)
                nc.vector.scalar_tensor_tensor(
                    out=h2[:], in0=h2[:], scalar=0.0,
                    in1=mn[:], op0=OP.max, op1=OP.add)
                nc.vector.tensor_scalar_sub(h2[:], h2[:], 1.0)
                # L2 node phase (bf16 transpose + W2 matmul)
                h2b = ppost.tile([128, fw], bf16, tag="h2b")
                nc.scalar.copy(h2b[:], h2[:])
                pst = ppsT.tile([128, TG, 128], bf16, tag="btp")
                nc.tensor.transpose(pst[:, 0, :], h2b[:], ident_bf[:])
                h2T = ppost.tile([128, 128], bf16, tag="h2T")
                nc.scalar.copy(h2T[:], pst[:, 0, :])
                ps2 = pps2.tile([128, n_classes + 2], f32, tag="hh")
                nc.tensor.matmul(ps2[:], lhsT=h2T[:], rhs=w2_sb[:],
                                 start=True, stop=True)
                l2r = ppost.tile([128, ROW2], f32, tag="l2r")
                nc.scalar.copy(l2r[:, 0:n_classes + 1],
                               ps2[:, 0:n_classes + 1])
                nc.vector.memset(l2r[:, n_classes + 1:ROW2], 0.0)
                nc.vector.tensor_copy(er2_hilo[:, k, 0:1],
                                      ps2[:, n_classes + 1:n_classes + 2])
                er2h = ppost.tile([128, 1], f32, tag="er2h")
                nc.vector.tensor_copy(er2h[:], er2_hilo[:, k, 0:1])
                nc.vector.tensor_tensor(
                    out=er2_hilo[:, k, 1:2],
                    in0=ps2[:, n_classes + 1:n_classes + 2],
                    in1=er2h[:], op=OP.subtract)
                nc.sync.dma_start(l2shard[k * CH:(k + 1) * CH, :], l2r[:])

            def post_chunk_l2(k, ps, ppost):
                fw = n_classes
                den = ppost.tile([128, 1], f32, tag="den2")
                nc.vector.tensor_scalar_max(den[:], ps[:, fw:fw + 1], 1e-30)
                rec = ppost.tile([128, 1], f32, tag="rec2")
                nc.vector.reciprocal(rec[:], den[:])
                xx = ppost.tile([128, fw], f32, tag="xx")
                nc.vector.tensor_scalar(out=xx[:], in0=ps[:, 0:fw],
                                        scalar1=rec[:], scalar2=None,
                                        op0=OP.mult)
                nc.vector.tensor_add(xx[:], xx[:], b2_sb[:])
                rmax = ppost.tile([128, 1], f32, tag="rmax")
                nc.vector.tensor_reduce(out=rmax[:], in_=xx[:],
                                        axis=AX.X, op=OP.max)
                nc.vector.tensor_scalar(out=xx_all[:, k, :], in0=xx[:],
                                        scalar1=rmax[:], scalar2=None,
                                        op0=OP.subtract)
                exs = ppost.tile([128, fw], f32, tag="exs")
                nc.scalar.activation(exs[:], xx_all[:, k, :], AF.Exp,
                                     accum_out=ssum_all[:, k:k + 1])

            def edge_phase(layer):
                if layer == 1:
                    tab, rw, fw, sw = htab, ROW1, hd, heads
                    gdt, erhl = bf16, er_hilo
                else:
                    tab, rw, fw, sw = l2tab, ROW2, n_classes, 1
                    gdt, erhl = f32, er2_hilo
                nw = fw + sw

                gblocks = {}
                for (b, w, g0, nt) in gath:
                    gblocks.setdefault(b, []).append((w, g0, nt))

                with ExitStack() as ectx:
                    pool = ectx.enter_context(
                        tc.tile_pool(name=f"edge{layer}", bufs=2))
                    pps = ectx.enter_context(
                        tc.tile_pool(name=f"eps{layer}", bufs=2,
                                     space="PSUM"))
                    ppsT = ectx.enter_context(
                        tc.tile_pool(name=f"epsT{layer}", bufs=2,
                                     space="PSUM"))
                    pper = ectx.enter_context(
                        tc.tile_pool(name=f"epsE{layer}", bufs=2,
                                     space="PSUM"))
                    ppost = ectx.enter_context(
                        tc.tile_pool(name=f"post{layer}", bufs=2))
                    pps2 = ectx.enter_context(
                        tc.tile_pool(name=f"ep2{layer}", bufs=2,
                                     space="PSUM")) if layer == 1 else None

                    def preamble(b):
                        segs = gblocks[b]
                        t0 = segs[0][1]
                        tb = sum(s[2] for s in segs)
                        gi = pool.tile([128, tb * 8], i16, tag="gi")
                        nc.sync.dma_start(gi[:],
                                          gidx_d[:, t0 * 8:(t0 + tb) * 8])
                        sl = pool.tile([128, tb], bf16, tag="sl")
                        nc.sync.dma_start(sl[:], slot_d[:, t0:t0 + tb])
                        gt = pool.tile([128, tb, rw], gdt, tag="gt")
                        for (w, g0, nt) in segs:
                            if nt == 0:
                                continue
                            wend = min(wbase[w] + WIN, np_nodes)
                            for s0 in range(0, nt, GMAX):
                                sn = min(GMAX, nt - s0)
                                off = g0 - t0 + s0
                                nc.gpsimd.dma_gather(
                                    out_ap=gt[:, off:off + sn, :],
                                    in_ap=tab[wbase[w]:wend, :],
                                    idxs_ap=gi[:, off * 8:(off + sn) * 8],
                                    num_idxs=sn * 128,
                                    num_idxs_reg=sn * 128, elem_size=rw,
                                    queue_num=0)
                        B = pool.tile([128, tb, 128], bf16, tag="B")
                        nc.vector.tensor_tensor(
                            out=B[:],
                            in0=iota_bf[:].unsqueeze(1)
                            .broadcast_to([128, tb, 128]),
                            in1=sl[:].unsqueeze(2)
                            .broadcast_to([128, tb, 128]),
                            op=OP.is_equal)
                        exw = pool.tile([128, tb, sw], f32, tag="exw")
                        for q0 in range(0, tb, ERG):
                            qn = min(ERG, tb - q0)
                            erp = pper.tile([128, ERG, 2 * sw], f32,
                                            tag="erps")
                            for c0 in range(q0, q0 + qn, TG):
                                cn = min(TG, q0 + qn - c0)
                                btp = ppsT.tile([128, TG, 128], bf16,
                                                tag="btp")
                                for i in range(cn):
                                    nc.tensor.transpose(
                                        btp[:, i, :], B[:, c0 + i, :],
                                        ident_bf[:])
                                bts = pool.tile([128, TG, 128], bf16,
                                                tag="bts")
                                nc.scalar.copy(bts[:, 0:cn, :],
                                               btp[:, 0:cn, :])
                                for i in range(cn):
                                    k = int(tile_chunk[t0 + c0 + i])
                                    nc.tensor.matmul(
                                        erp[:, c0 + i - q0, :],
                                        lhsT=bts[:, i, :],
                                        rhs=erhl[:, k, :],
                                        start=True, stop=True)
                            nc.vector.tensor_copy(exw[:, q0:q0 + qn, :],
                                                  erp[:, 0:qn, 0:sw])
                            nc.vector.tensor_add(exw[:, q0:q0 + qn, :],
                                                 exw[:, q0:q0 + qn, :],
                                                 erp[:, 0:qn, sw:2 * sw])
                        return dict(b=b, t0=t0, tb=tb, gt=gt, B=B, exw=exw)

                    def body(st):
                        b, t0, tb = st["b"], st["t0"], st["tb"]
                        gt, B, exw = st["gt"], st["B"], st["exw"]
                        if layer == 1:
                            el_ap = gt[:, :, fw:fw + 2 * sw].bitcast(f32)
                        else:
                            el_ap = gt[:, :, fw:fw + sw]
                        nc.vector.tensor_add(exw[:], exw[:], el_ap)
                        nc.vector.scalar_tensor_tensor(
                            out=exw[:], in0=exw[:], scalar=NEG_SLOPE,
                            in1=exw[:], op0=OP.mult, op1=OP.max)
                        comb = pool.tile([128, tb, nw], bf16, tag="comb")
                        nc.scalar.activation(comb[:, :, fw:fw + sw], exw[:],
                                             AF.Exp)
                        nc.vector.tensor_mul(
                            comb[:, :, 0:fw].rearrange(
                                "p t (s d) -> p t s d", s=sw),
                            gt[:, :, 0:fw].rearrange(
                                "p t (s d) -> p t s d", s=sw),
                            comb[:, :, fw:fw + sw].unsqueeze(3)
                            .broadcast_to([128, tb, sw, fw // sw]))
                        for k in range(b * BLK, min((b + 1) * BLK, nchunk)):
                            tl = chunk_tiles[k]
                            ps = pps.tile([128, nw], f32, tag="agg")
                            for j, t in enumerate(tl):
                                nc.tensor.matmul(
                                    ps[:], lhsT=B[:, t - t0, :],
                                    rhs=comb[:, t - t0, :],
                                    start=(j == 0), stop=(j == len(tl) - 1))
                            if layer == 1:
                                post_chunk_l1(k, ps, ppost, ppsT, pps2)
                            else:
                                post_chunk_l2(k, ps, ppost)
                        if layer == 1 and l2_split_blk is not None \
                                and b == l2_split_blk:
                            nc.gpsimd.collective_compute(
                                "AllGather", OP.bypass,
                                replica_groups=replica,
                                ins=[l2shard[0:h0, :]],
                                outs=[l2tab[0:NC * h0, :]])

                    st = preamble(0)
                    for b in range(1, nblk):
                        st2 = preamble(b)
                        body(st)
                        st = st2
                    body(st)

            edge_phase(1)
            if l2_split_blk is not None:
                nc.gpsimd.collective_compute(
                    "AllGather", OP.bypass, replica_groups=replica,
                    ins=[l2shard[h0:sh, :]],
                    outs=[l2tab[NC * h0:np_nodes, :]])
            else:
                nc.gpsimd.collective_compute(
                    "AllGather", OP.bypass, replica_groups=replica,
                    ins=[l2shard[:]], outs=[l2tab[:]])
            edge_phase(2)

            # two-pass log-softmax epilogue
            nc.scalar.activation(lss_all[:], ssum_all[:], AF.Ln)
            nc.vector.tensor_tensor(
                out=xx_all[:], in0=xx_all[:],
                in1=lss_all[:].unsqueeze(2)
                .broadcast_to([128, nchunk, n_classes]),
                op=OP.subtract)
            nc.sync.dma_start(
                out_d[:].rearrange("(k p) c -> p k c", p=128),
                xx_all[:])

    nc_.compile()
    return nc_


# ------------------------------------------------------------------ driver
def make_in_maps(meta, feat, W1, al1, ar1, b1, W2, al2, ar2, b2):
    sh, np_nodes = meta["sh"], meta["np_nodes"]
    n_real, f_in = feat.shape
    kt = f_in // 128
    hd = W1.shape[1]
    heads, hidd = al1.shape if al1.ndim == 2 else (1, al1.shape[0])
    W1al = np.einsum("fhd,hd->fh", W1.reshape(f_in, heads, hidd), al1)
    W1ar = np.einsum("fhd,hd->fh", W1.reshape(f_in, heads, hidd), ar1)
    W1e = np.concatenate([W1, W1al, W1ar], axis=1)          # [f_in, we]
    we = W1e.shape[1]
    W1er = np.ascontiguousarray(
        W1e.reshape(kt, 128, we).transpose(1, 0, 2)).astype(BF16)
    b1rep = np.tile(b1.reshape(1, -1), (128, 1)).astype(np.float32)
    W2a = np.concatenate(
        [W2, W2 @ al2.reshape(-1, 1), W2 @ ar2.reshape(-1, 1)],
        axis=1).astype(BF16)
    b2rep = np.tile(b2.reshape(1, -1), (128, 1)).astype(np.float32)

    featP = np.zeros((np_nodes, f_in), dtype=np.float32)
    featP[:n_real] = feat
    fpi = featP[meta["row_node"]]                            # pi-ordered
    featT = np.ascontiguousarray(
        fpi.T.reshape(kt, 128, np_nodes).transpose(1, 0, 2)).astype(BF16)

    in_maps = []
    for c in range(NC):
        fo = featP[c * sh:(c + 1) * sh]
        featO = np.ascontiguousarray(
            fo.T.reshape(kt, 128, sh).transpose(1, 0, 2)).astype(BF16)
        in_maps.append({
            "featT": featT, "featO": featO,
            "W1e": W1er, "b1rep": b1rep,
            "W2a": W2a, "b2rep": b2rep,
            "gidx": _wrap16(meta["gidx"][c]),
            "slot": np.ascontiguousarray(
                meta["slot"][c].reshape(-1, 128).T).astype(BF16),
        })
    return in_maps


class Runner:
    """Builds the SPMD program once; exposes a repeatable timed executor."""

    def __init__(self, meta, f_in):
        self.meta = meta
        self.nc = build_program(meta, f_in, HID, HEADS, N_CLASSES)
        self._fn = None
        self.repeat = 1

    repeat = 1

    def _lower(self):
        import jax
        import numpy as _np
        from jax.sharding import Mesh, PartitionSpec
        from jax.experimental.shard_map import shard_map
        from concourse import mybir
        from concourse.bass2jax import _bass_exec_p, install_neuronx_cc_hook

        install_neuronx_cc_hook()
        nc = self.nc
        in_names, out_names, out_avals, zero_outs = [], [], [], []
        partition_name = (nc.partition_id_tensor.name
                          if nc.partition_id_tensor else None)
        for alloc in nc.m.functions[0].allocations:
            if not isinstance(alloc, mybir.MemoryLocationSet):
                continue
            name = alloc.memorylocations[0].name
            if alloc.kind == "ExternalInput":
                if name != partition_name:
                    in_names.append(name)
            elif alloc.kind == "ExternalOutput":
                shape = tuple(alloc.tensor_shape)
                dtype = mybir.dt.np(alloc.dtype)
                out_names.append(name)
                out_avals.append(jax.core.ShapedArray(shape, dtype))
                zero_outs.append(_np.zeros(shape, dtype))
        n_params = len(in_names)
        n_outs = len(out_avals)
        all_in_names = list(in_names) + list(out_names)
        if partition_name is not None:
            all_in_names.append(partition_name)

        repeat = self.repeat

        def _body(*args):
            ins = list(args[:n_params])
            zouts = list(args[n_params:])
            outs = None
            for _ in range(repeat):
                operands = ins + zouts
                if partition_name is not None:
                    from concourse.bass2jax import partition_id_tensor
                    operands.append(partition_id_tensor())
                outs = _bass_exec_p.bind(
                    *operands, out_avals=tuple(out_avals),
                    in_names=tuple(all_in_names), out_names=tuple(out_names),
                    lowering_input_output_aliases=(),
                    sim_require_finite=False, sim_require_nnan=False, nc=nc)
                zouts = list(outs)   # chain: serialize + reuse as out bufs
            return tuple(outs)

        devices = jax.devices()[:NC]
        mesh = Mesh(_np.asarray(devices), ("core",))
        in_specs = (PartitionSpec("core"),) * (n_params + n_outs)
        out_specs = (PartitionSpec("core"),) * n_outs
        self._fn = jax.jit(
            shard_map(_body, mesh=mesh, in_specs=in_specs,
                      out_specs=out_specs, check_rep=False),
            keep_unused=True)
        self._in_names = in_names
        self._out_names = out_names
        self._out_avals = out_avals
        self._zero_outs = zero_outs
        self._mesh = mesh
        self._in_specs = in_specs

    def prepare(self, in_maps):
        import jax
        import numpy as _np
        from jax.sharding import NamedSharding, PartitionSpec
        if self._fn is None:
            self._lower()
        concat_in = [
            _np.concatenate([in_maps[c][name] for c in range(NC)], axis=0)
            for name in self._in_names]
        concat_zeros = [
            _np.zeros((NC * z.shape[0], *z.shape[1:]), z.dtype)
            for z in self._zero_outs]
        shd = NamedSharding(self._mesh, PartitionSpec("core"))
        self._args = [jax.device_put(a, shd) for a in concat_in + concat_zeros]
        jax.block_until_ready(self._args)

    def run(self):
        import jax
        out = self._fn(*self._args)
        out = jax.block_until_ready(out)
        import numpy as _np
        res = _np.asarray(out[self._out_names.index("out")])
        sh = self._out_avals[self._out_names.index("out")].shape
        return res.reshape(NC, *sh).reshape(NC * sh[0], *sh[1:])


_RUNNER = None


def get_runner(feat, src, dst):
    global _RUNNER
    n, f_in = feat.shape
    meta = host_prep(np.asarray(src, np.int32), np.asarray(dst, np.int32),
                     n_nodes=n)
    _RUNNER = Runner(meta, f_in)
    return _RUNNER


def kernel(feat, src, dst, W1, al1, ar1, b1, W2, al2, ar2, b2):
    feat = np.asarray(feat, dtype=np.float32)
    src = np.asarray(src, dtype=np.int32)
    dst = np.asarray(dst, dtype=np.int32)
    args = [np.asarray(x, np.float32)
            for x in (W1, al1, ar1, b1, W2, al2, ar2, b2)]
    r = _RUNNER if _RUNNER is not None else get_runner(feat, src, dst)
    in_maps = make_in_maps(r.meta, feat, *args)
    r.prepare(in_maps)
    return r.run()[:feat.shape[0]]


kernel.last_exec_time_ns = None


# revision 4
# speedup vs baseline: 3.8976x; 3.8178x over previous
"""Bass/Trainium2 kernel for a 2-layer GAT (GATConv x2 + log_softmax) on 8 NeuronCores.

v3 design (edge/data parallel, dst-sharded, replicated node phase):
  - Shards padded to sh=ceil(N/NC/128)*128 nodes; core c owns dst nodes
    [c*sh, (c+1)*sh).  Table rows permuted (pi: half0 of every core first,
    then half1) so the l2tab AllGather can be emitted in 2 contiguous
    halves, the first overlapping the L1 edge-phase tail.
  - Phase A is REPLICATED: every core computes h|el for ALL nodes from the
    full (pi-ordered) feature matrix and writes its own htab copy -> no
    htab collective at all.  el/er come free out of the PE via host-extended
    weights [W1 | W1@al | W1@ar].  A tiny per-core pass over the core's own
    shard produces er (er_hilo, exact bf16 hi+lo pair, kept in SBUF).
  - Edge phase (dst-chunk blocked, software-pipelined preamble/body so
    in-order engines don't convoy): dma_gather h[src] 512B rows from htab;
    one-hot B[edge,slot] built on DVE; er[dst] per edge via PE transpose of
    B + matmul against er_hilo (no er gather, no didx); ex=exp(leaky(el+er));
    aggregate with PE matmuls psum += B^T @ [ex*h | ex]; normalize, bias,
    ELU -> fused L2 node phase (bf16 transpose + W2a matmul) -> l2shard.
  - L2 edge phase: same pipeline on 256B f32 rows; log_softmax two-pass
    (exp per chunk, single Ln at the end); one bulk output DMA.
"""

import os
import sys

import numpy as np

sys.path.insert(0, "/opt/trn_rl_repo")

import ml_dtypes

BF16 = ml_dtypes.bfloat16

# ---------------------------------------------------------------- constants
N_NODES = 100000
F_IN = 256
HID = 16
HEADS = 8
N_CLASSES = 16
NEG_SLOPE = 0.2
NC = 8                      # cores
CH = 128                    # dst nodes per chunk
BLK = 2                     # chunks per block (gather granularity)
GMAX = 8                    # max 128-edge tiles per dma_gather instruction
RING = 16384                # dynamic dma scratch bytes (1024-desc ring)
ROW1 = 256                  # bf16 per L1 table row (h 128 | el 16 | pad)
ROW2 = 64                   # f32 per L2 table row (hh 16 | el2 1 | pad)
WIN = 32768                 # rows per int16 gather window
ERG = 16                    # tiles per er-lookup psum group
TG = 4                      # tiles per transpose/copy group


def _wrap16(v):
    # [n] -> [128, n//16] int16; idx i at [i%16, i//16], replicated over groups
    n = v.shape[0]
    assert n % 16 == 0
    a = v.reshape(n // 16, 16).T.astype(np.int16)      # [16, n//16]
    return np.ascontiguousarray(np.tile(a, (8, 1)))    # [128, n//16]


def host_prep(src, dst, n_nodes=N_NODES, nc=NC, ch=CH):
    """Partition/sort/pad edges; build gather index + slot arrays per core."""
    sh = ((n_nodes + nc - 1) // nc + 127) // 128 * 128
    np_nodes = nc * sh
    ntile_a = sh // 128
    nchunk = sh // ch
    # split the shard into NG chunk-aligned groups; the l2tab AllGather is
    # emitted per group so all but the last overlap the L1 edge tail.
    NG = 4
    ngrp = max(1, min(NG, nchunk // 8))
    base, rem = nchunk // ngrp, nchunk % ngrp
    sizes = [base + 1] * rem + [base] * (ngrp - rem)
    P = np.concatenate([[0], np.cumsum(np.asarray(sizes) * ch)]).astype(
        np.int64)                        # within-shard row starts, P[-1]=sh

    def pi(n):
        c, r = n // sh, n % sh
        g = np.searchsorted(P, r, side="right") - 1
        return nc * P[g] + c * (P[g + 1] - P[g]) + (r - P[g])
    psrc = pi(src.astype(np.int64))
    nwin = (np_nodes + WIN - 1) // WIN
    bases = [w * WIN for w in range(nwin)]

    per_core = []
    cnt = np.zeros((nc, nchunk, nwin), dtype=np.int64)
    for c in range(nc):
        m = (dst // sh) == c
        es, ed = psrc[m], (dst[m] - c * sh).astype(np.int64)
        chunk = ed // ch
        bank = es // WIN
        order = np.lexsort((bank, chunk))
        es, ed, chunk, bank = es[order], ed[order], chunk[order], bank[order]
        per_core.append((es, ed))
        for k in range(nchunk):
            km = chunk == k
            for w in range(nwin):
                cnt[c, k, w] = np.count_nonzero(km & (bank == w))

    # uniform tile counts: max over cores, ceil to 128-edge tiles
    T = np.maximum(np.ceil(cnt.max(axis=0) / 128.0), 1).astype(np.int64)

    nblk = (nchunk + BLK - 1) // BLK
    # stream layout: for blk: for w: for k in blk: seg(k, w) of T[k,w] tiles
    seg_off = {}
    gath = []             # (block, window, tile_off, ntiles)
    toff = 0
    for b in range(nblk):
        ks = list(range(b * BLK, min((b + 1) * BLK, nchunk)))
        for w in range(nwin):
            g0 = toff
            for k in ks:
                seg_off[(k, w)] = toff
                toff += int(T[k, w])
            gath.append((b, w, g0, toff - g0))
    ntile = toff
    ne_pad = ntile * 128

    gidx = np.zeros((nc, ne_pad), dtype=np.int16)
    slot = np.full((nc, ne_pad), -1, dtype=np.float32)
    for c in range(nc):
        es, ed = per_core[c]
        p = 0
        ptr = {}
        for k in range(nchunk):
            for w in range(nwin):
                n = int(cnt[c, k, w])
                ptr[(k, w)] = (p, p + n)
                p += n
        for (k, w), off in seg_off.items():
            a, bnd = ptr[(k, w)]
            n = bnd - a
            pos = off * 128
            gidx[c, pos:pos + n] = (es[a:bnd] - bases[w]).astype(np.int16)
            slot[c, pos:pos + n] = (ed[a:bnd] % ch).astype(np.float32)

    chunk_tiles = []
    tile_chunk = np.zeros(ntile, dtype=np.int64)
    for k in range(nchunk):
        tl = []
        for w in range(nwin):
            off = seg_off[(k, w)]
            tl.extend(range(off, off + int(T[k, w])))
        chunk_tiles.append(tl)
        for t in tl:
            tile_chunk[t] = k

    # l2 AllGather emission: group g is ready after the block finishing its
    # last chunk; groups landing on the final block are emitted after L1.
    ag_blk = []
    ends = np.cumsum(sizes)
    for g in range(ngrp):
        blk = (int(ends[g]) - 1) // BLK
        ag_blk.append(blk if g < ngrp - 1 and blk < nblk - 1 else None)

    # table row -> node id (for building the pi-ordered feature matrix)
    j = np.arange(np_nodes, dtype=np.int64)
    gP = nc * P                          # table row starts per group
    gj = np.searchsorted(gP, j, side="right") - 1
    gj = np.minimum(gj, ngrp - 1)
    rows_g = (P[gj + 1] - P[gj])
    jj = j - gP[gj]
    row_node = (jj // rows_g) * sh + P[gj] + jj % rows_g

    return dict(
        n_nodes=n_nodes, np_nodes=np_nodes, sh=sh, nchunk=nchunk, nwin=nwin,
        bases=bases, nblk=nblk, ntile=ntile, ne_pad=ne_pad, gath=gath,
        chunk_tiles=chunk_tiles, tile_chunk=tile_chunk, seg_off=seg_off, T=T,
        gidx=gidx, slot=slot, P=P, ngrp=ngrp, ag_blk=ag_blk,
        row_node=row_node,
    )


# ------------------------------------------------------------- bass program
def build_program(meta, f_in, hid, heads, n_classes):
    from contextlib import ExitStack

    import concourse.tile as tile
    from concourse import bacc, mybir

    dt = mybir.dt
    f32, bf16, i16 = dt.float32, dt.bfloat16, dt.int16
    AF = mybir.ActivationFunctionType
    OP = mybir.AluOpType
    AX = mybir.AxisListType

    np_nodes = meta["np_nodes"]
    sh, nchunk, nwin = meta["sh"], meta["nchunk"], meta["nwin"]
    nblk, ntile, ne_pad = meta["nblk"], meta["ntile"], meta["ne_pad"]
    gath, chunk_tiles = meta["gath"], meta["chunk_tiles"]
    tile_chunk = meta["tile_chunk"]
    wbase = meta["bases"]
    P, ngrp, ag_blk = meta["P"], meta["ngrp"], meta["ag_blk"]
    hd = heads * hid
    kt = f_in // 128
    we = hd + 2 * heads            # extended W1 columns: h | el | er
    ntile_a = sh // 128

    nc_ = bacc.Bacc("TRN2", target_bir_lowering=False, debug=False,
                    num_devices=NC, dynamic_dma_scratch_size=RING)

    def din(name, shape, dtype):
        return nc_.dram_tensor(name, list(shape), dtype,
                               kind="ExternalInput").ap()

    featT = din("featT", [128, kt, np_nodes], bf16)   # full, pi-ordered
    featO = din("featO", [128, kt, sh], bf16)         # own shard, natural
    W1e = din("W1e", [128, kt, we], bf16)
    b1rep = din("b1rep", [128, hd], f32)
    W2a = din("W2a", [hd, n_classes + 2], bf16)
    b2rep = din("b2rep", [128, n_classes], f32)
    gidx_d = din("gidx", [128, ne_pad // 16], i16)
    slot_d = din("slot", [128, ntile], bf16)
    out_d = nc_.dram_tensor("out", [sh, n_classes], f32,
                            kind="ExternalOutput").ap()

    htab = nc_.dram_tensor("htab", [np_nodes, ROW1], bf16).ap()
    l2tab = nc_.dram_tensor("l2tab", [np_nodes, ROW2], f32).ap()
    l2shard = nc_.dram_tensor("l2shard", [sh, ROW2], f32).ap()

    replica = [list(range(NC))]

    with tile.TileContext(nc_) as tc:
        nc = tc.nc
        with ExitStack() as cctx:
            cpool = cctx.enter_context(tc.tile_pool(name="const", bufs=1))
            w1_sb = cpool.tile([128, kt * we], bf16, tag="w1")
            nc.sync.dma_start(
                w1_sb[:].rearrange("p (k o) -> p k o", k=kt), W1e[:])
            b1_sb = cpool.tile([128, hd], f32, tag="b1")
            nc.sync.dma_start(b1_sb[:], b1rep[:])
            w2_sb = cpool.tile([hd, n_classes + 2], bf16, tag="w2")
            nc.sync.dma_start(w2_sb[:], W2a[:])
            b2_sb = cpool.tile([128, n_classes], f32, tag="b2")
            nc.sync.dma_start(b2_sb[:], b2rep[:])
            iota_sb = cpool.tile([128, 128], f32, tag="iota")
            nc.gpsimd.iota(iota_sb[:], pattern=[[1, 128]], base=0,
                           channel_multiplier=0,
                           allow_small_or_imprecise_dtypes=True)
            iota_p = cpool.tile([128, 1], f32, tag="iotap")
            nc.gpsimd.iota(iota_p[:], pattern=[[0, 1]], base=0,
                           channel_multiplier=1,
                           allow_small_or_imprecise_dtypes=True)
            ident_sb = cpool.tile([128, 128], f32, tag="ident")
            nc.vector.tensor_scalar(out=ident_sb[:], in0=iota_sb[:],
                                    scalar1=iota_p[:], scalar2=None,
                                    op0=OP.is_equal)
            iota_bf = cpool.tile([128, 128], bf16, tag="iotabf")
            nc.vector.tensor_copy(iota_bf[:], iota_sb[:])
            ident_bf = cpool.tile([128, 128], bf16, tag="identbf")
            nc.vector.tensor_copy(ident_bf[:], ident_sb[:])

            er_hilo = cpool.tile([128, nchunk, 2 * heads], bf16, tag="erhl")
            nc.vector.memset(er_hilo[:], 0.0)
            er2_hilo = cpool.tile([128, nchunk, 2], bf16, tag="er2hl")
            nc.vector.memset(er2_hilo[:], 0.0)
            xx_all = cpool.tile([128, nchunk, n_classes], f32, tag="xxall")
            ssum_all = cpool.tile([128, nchunk], f32, tag="ssall")
            nc.vector.memset(ssum_all[:], 1.0)
            lss_all = cpool.tile([128, nchunk], f32, tag="lssall")

            # ---------------- phase A0: own-shard er ----------------
            with ExitStack() as octx:
                opool = octx.enter_context(tc.tile_pool(name="phO", bufs=2))
                opsum = octx.enter_context(
                    tc.tile_pool(name="phOps", bufs=2, space="PSUM"))
                for t0_ in range(0, ntile_a, 4):
                    tn = min(4, ntile_a - t0_)
                    fo = opool.tile([128, kt, 4 * 128], bf16, tag="fo")
                    nc.sync.dma_start(
                        fo[:, :, :tn * 128],
                        featO[:, :, t0_ * 128:(t0_ + tn) * 128])
                    pso = opsum.tile([128, 4, 2 * heads], f32, tag="pso")
                    for j in range(tn):
                        for k in range(kt):
                            nc.tensor.matmul(
                                pso[:, j, :],
                                lhsT=fo[:, k, j * 128:(j + 1) * 128],
                                rhs=w1_sb[:, k * we + hd:(k + 1) * we],
                                start=(k == 0), stop=(k == kt - 1))
                    # er hi/lo (exact bf16 pair); er lives in cols heads:2*heads
                    nc.vector.tensor_copy(
                        er_hilo[:, t0_:t0_ + tn, 0:heads],
                        pso[:, 0:tn, heads:2 * heads])
                    hi_f = opool.tile([128, 4, heads], f32, tag="hif")
                    nc.vector.tensor_copy(
                        hi_f[:, 0:tn, :], er_hilo[:, t0_:t0_ + tn, 0:heads])
                    nc.vector.tensor_tensor(
                        out=er_hilo[:, t0_:t0_ + tn, heads:2 * heads],
                        in0=pso[:, 0:tn, heads:2 * heads],
                        in1=hi_f[:, 0:tn, :],
                        op=OP.subtract)

            # ---------------- phase A: replicated h|el sweep ----------------
            with ExitStack() as actx:
                apool = actx.enter_context(tc.tile_pool(name="phA", bufs=3))
                apsum = actx.enter_context(
                    tc.tile_pool(name="phAps", bufs=4, space="PSUM"))
                for t0_ in range(0, np_nodes // 128, 4):
                    ft = apool.tile([128, kt, 4 * 128], bf16, tag="ft")
                    nc.sync.dma_start(
                        ft[:, :, :], featT[:, :, t0_ * 128:(t0_ + 4) * 128])
                    row = apool.tile([128, 4, ROW1], bf16, tag="row")
                    for j in range(4):
                        ps = apsum.tile([128, we], f32, tag="hps")
                        for k in range(kt):
                            nc.tensor.matmul(
                                ps[:, :],
                                lhsT=ft[:, k, j * 128:(j + 1) * 128],
                                rhs=w1_sb[:, k * we:(k + 1) * we],
                                start=(k == 0), stop=(k == kt - 1))
                        nc.scalar.copy(row[:, j, 0:hd], ps[:, 0:hd])
                        nc.vector.tensor_copy(
                            row[:, j, hd:hd + 2 * heads].bitcast(f32),
                            ps[:, hd:hd + heads])
                    nc.vector.memset(row[:, :, hd + 2 * heads:ROW1], 0.0)
                    nc.sync.dma_start(
                        htab[t0_ * 128:(t0_ + 4) * 128, :].rearrange(
                            "(j p) r -> p j r", p=128),
                        row[:])

            def emit_l2_ag(g):
                r0, r1 = int(P[g]), int(P[g + 1])
                nc.gpsimd.collective_compute(
                    "AllGather", OP.bypass, replica_groups=replica,
                    ins=[l2shard[r0:r1, :]],
                    outs=[l2tab[NC * r0:NC * r1, :]])

            # ---------------- edge phases ----------------
            def post_chunk_l1(k, ps, ppost, ppsT, pps2):
                fw, sw = hd, heads
                den = ppost.tile([128, sw], f32, tag="den")
                nc.vector.tensor_scalar_max(den[:], ps[:, fw:fw + sw], 1e-30)
                rec = ppost.tile([128, sw], f32, tag="rec")
                nc.vector.reciprocal(rec[:], den[:])
                h2 = ppost.tile([128, fw], f32, tag="h2")
                nc.vector.tensor_mul(
                    h2[:].rearrange("p (s d) -> p s d", s=sw),
                    ps[:, 0:fw].rearrange("p (s d) -> p s d", s=sw),
                    rec[:].unsqueeze(2).broadcast_to([128, sw, fw // sw]))
                nc.vector.tensor_add(h2[:], h2[:], b1_sb[:])
                mn = ppost.tile([128, fw], f32, tag="mn")
                nc.vector.tensor_scalar_min(mn[:], h2[:], 0.0)
                nc.scalar.activation(mn[:], mn[:], AF.Exp)
                nc.vector.scalar_tensor_tensor(
                    out=h2[:], in0=h2[:], scalar=0.0,
                    in1=mn[:], op0=OP.max, op1=OP.add)
                nc.vector.tensor_scalar_sub(h2[:], h2[:], 1.0)
                # L2 node phase (bf16 transpose + W2 matmul)
                h2b = ppost.tile([128, fw], bf16, tag="h2b")
                nc.scalar.copy(h2b[:], h2[:])
                pst = ppsT.tile([128, TG, 128], bf16, tag="btp")
                nc.tensor.transpose(pst[:, 0, :], h2b[:], ident_bf[:])
                h2T = ppost.tile([128, 128], bf16, tag="h2T")
                nc.scalar.copy(h2T[:], pst[:, 0, :])
                ps2 = pps2.tile([128, n_classes + 2], f32, tag="hh")
                nc.tensor.matmul(ps2[:], lhsT=h2T[:], rhs=w2_sb[:],
                                 start=True, stop=True)
                l2r = ppost.tile([128, ROW2], f32, tag="l2r")
                nc.scalar.copy(l2r[:, 0:n_classes + 1],
                               ps2[:, 0:n_classes + 1])
                nc.vector.memset(l2r[:, n_classes + 1:ROW2], 0.0)
                nc.vector.tensor_copy(er2_hilo[:, k, 0:1],
                                      ps2[:, n_classes + 1:n_classes + 2])
                er2h = ppost.tile([128, 1], f32, tag="er2h")
                nc.vector.tensor_copy(er2h[:], er2_hilo[:, k, 0:1])
                nc.vector.tensor_tensor(
                    out=er2_hilo[:, k, 1:2],
                    in0=ps2[:, n_classes + 1:n_classes + 2],
                    in1=er2h[:], op=OP.subtract)
                nc.sync.dma_start(l2shard[k * CH:(k + 1) * CH, :], l2r[:])

            def post_chunk_l2(k, ps, ppost):
                fw = n_classes
                den = ppost.tile([128, 1], f32, tag="den2")
                nc.vector.tensor_scalar_max(den[:], ps[:, fw:fw + 1], 1e-30)
                rec = ppost.tile([128, 1], f32, tag="rec2")
                nc.vector.reciprocal(rec[:], den[:])
                xx = ppost.tile([128, fw], f32, tag="xx")
                nc.vector.tensor_scalar(out=xx[:], in0=ps[:, 0:fw],
                                        scalar1=rec[:], scalar2=None,
                                        op0=OP.mult)
                nc.vector.tensor_add(xx[:], xx[:], b2_sb[:])
                rmax = ppost.tile([128, 1], f32, tag="rmax")
                nc.vector.tensor_reduce(out=rmax[:], in_=xx[:],
                                        axis=AX.X, op=OP.max)
                nc.vector.tensor_scalar(out=xx_all[:, k, :], in0=xx[:],
                                        scalar1=rmax[:], scalar2=None,
                                        op0=OP.subtract)
                exs = ppost.tile([128, fw], f32, tag="exs")
                nc.scalar.activation(exs[:], xx_all[:, k, :], AF.Exp,
                                     accum_out=ssum_all[:, k:k + 1])

            def edge_phase(layer):
                if layer == 1:
                    tab, rw, fw, sw = htab, ROW1, hd, heads
                    gdt, erhl = bf16, er_hilo
                else:
                    tab, rw, fw, sw = l2tab, ROW2, n_classes, 1
                    gdt, erhl = f32, er2_hilo
                nw = fw + sw

                gblocks = {}
                for (b, w, g0, nt) in gath:
                    gblocks.setdefault(b, []).append((w, g0, nt))

                with ExitStack() as ectx:
                    pool = ectx.enter_context(
                        tc.tile_pool(name=f"edge{layer}", bufs=2))
                    pps = ectx.enter_context(
                        tc.tile_pool(name=f"eps{layer}", bufs=2,
                                     space="PSUM"))
                    ppsT = ectx.enter_context(
                        tc.tile_pool(name=f"epsT{layer}", bufs=2,
                                     space="PSUM"))
                    pper = ectx.enter_context(
                        tc.tile_pool(name=f"epsE{layer}", bufs=2,
                                     space="PSUM"))
                    ppost = ectx.enter_context(
                        tc.tile_pool(name=f"post{layer}", bufs=2))
                    pps2 = ectx.enter_context(
                        tc.tile_pool(name=f"ep2{layer}", bufs=2,
                                     space="PSUM")) if layer == 1 else None

                    def preamble(b):
                        segs = gblocks[b]
                        t0 = segs[0][1]
                        tb = sum(s[2] for s in segs)
                        gi = pool.tile([128, tb * 8], i16, tag="gi")
                        nc.sync.dma_start(gi[:],
                                          gidx_d[:, t0 * 8:(t0 + tb) * 8])
                        sl = pool.tile([128, tb], bf16, tag="sl")
                        nc.sync.dma_start(sl[:], slot_d[:, t0:t0 + tb])
                        gt = pool.tile([128, tb, rw], gdt, tag="gt")
                        for (w, g0, nt) in segs:
                            if nt == 0:
                                continue
                            wend = min(wbase[w] + WIN, np_nodes)
                            for s0 in range(0, nt, GMAX):
                                sn = min(GMAX, nt - s0)
                                off = g0 - t0 + s0
                                nc.gpsimd.dma_gather(
                                    out_ap=gt[:, off:off + sn, :],
                                    in_ap=tab[wbase[w]:wend, :],
                                    idxs_ap=gi[:, off * 8:(off + sn) * 8],
                                    num_idxs=sn * 128,
                                    num_idxs_reg=sn * 128, elem_size=rw,
                                    queue_num=0)
                        B = pool.tile([128, tb, 128], bf16, tag="B")
                        nc.vector.tensor_tensor(
                            out=B[:],
                            in0=iota_bf[:].unsqueeze(1)
                            .broadcast_to([128, tb, 128]),
                            in1=sl[:].unsqueeze(2)
                            .broadcast_to([128, tb, 128]),
                            op=OP.is_equal)
                        exw = pool.tile([128, tb, sw], f32, tag="exw")
                        for q0 in range(0, tb, ERG):
                            qn = min(ERG, tb - q0)
                            erp = pper.tile([128, ERG, 2 * sw], f32,
                                            tag="erps")
                            for c0 in range(q0, q0 + qn, TG):
                                cn = min(TG, q0 + qn - c0)
                                btp = ppsT.tile([128, TG, 128], bf16,
                                                tag="btp")
                                for i in range(cn):
                                    nc.tensor.transpose(
                                        btp[:, i, :], B[:, c0 + i, :],
                                        ident_bf[:])
                                bts = pool.tile([128, TG, 128], bf16,
                                                tag="bts")
                                nc.scalar.copy(bts[:, 0:cn, :],
                                               btp[:, 0:cn, :])
                                for i in range(cn):
                                    k = int(tile_chunk[t0 + c0 + i])
                                    nc.tensor.matmul(
                                        erp[:, c0 + i - q0, :],
                                        lhsT=bts[:, i, :],
                                        rhs=erhl[:, k, :],
                                        start=True, stop=True)
                            nc.vector.tensor_copy(exw[:, q0:q0 + qn, :],
                                                  erp[:, 0:qn, 0:sw])
                            nc.vector.tensor_add(exw[:, q0:q0 + qn, :],
                                                 exw[:, q0:q0 + qn, :],
                                                 erp[:, 0:qn, sw:2 * sw])
                        return dict(b=b, t0=t0, tb=tb, gt=gt, B=B, exw=exw)

                    def body(st):
                        b, t0, tb = st["b"], st["t0"], st["tb"]
                        gt, B, exw = st["gt"], st["B"], st["exw"]
                        if layer == 1:
                            el_ap = gt[:, :, fw:fw + 2 * sw].bitcast(f32)
                        else:
                            el_ap = gt[:, :, fw:fw + sw]
                        nc.vector.tensor_add(exw[:], exw[:], el_ap)
                        nc.vector.scalar_tensor_tensor(
                            out=exw[:], in0=exw[:], scalar=NEG_SLOPE,
                            in1=exw[:], op0=OP.mult, op1=OP.max)
                        comb = pool.tile([128, tb, nw], bf16, tag="comb")
                        nc.scalar.activation(comb[:, :, fw:fw + sw], exw[:],
                                             AF.Exp)
                        nc.vector.tensor_mul(
                            comb[:, :, 0:fw].rearrange(
                                "p t (s d) -> p t s d", s=sw),
                            gt[:, :, 0:fw].rearrange(
                                "p t (s d) -> p t s d", s=sw),
                            comb[:, :, fw:fw + sw].unsqueeze(3)
                            .broadcast_to([128, tb, sw, fw // sw]))
                        for k in range(b * BLK, min((b + 1) * BLK, nchunk)):
                            tl = chunk_tiles[k]
                            ps = pps.tile([128, nw], f32, tag="agg")
                            for j, t in enumerate(tl):
                                nc.tensor.matmul(
                                    ps[:], lhsT=B[:, t - t0, :],
                                    rhs=comb[:, t - t0, :],
                                    start=(j == 0), stop=(j == len(tl) - 1))
                            if layer == 1:
                                post_chunk_l1(k, ps, ppost, ppsT, pps2)
                            else:
                                post_chunk_l2(k, ps, ppost)
                        if layer == 1:
                            for g in range(ngrp):
                                if ag_blk[g] == b:
                                    emit_l2_ag(g)

                    st = preamble(0)
                    for b in range(1, nblk):
                        st2 = preamble(b)
                        body(st)
                        st = st2
                    body(st)

            edge_phase(1)
            for g in range(ngrp):
                if ag_blk[g] is None:
                    emit_l2_ag(g)
            edge_phase(2)

            # two-pass log-softmax epilogue
            nc.scalar.activation(lss_all[:], ssum_all[:], AF.Ln)
            nc.vector.tensor_tensor(
                out=xx_all[:], in0=xx_all[:],
                in1=lss_all[:].unsqueeze(2)
                .broadcast_to([128, nchunk, n_classes]),
                op=OP.subtract)
            nc.sync.dma_start(
                out_d[:].rearrange("(k p) c -> p k c", p=128),
                xx_all[:])

    nc_.compile()
    return nc_


# ------------------------------------------------------------------ driver
def make_in_maps(meta, feat, W1, al1, ar1, b1, W2, al2, ar2, b2):
    sh, np_nodes = meta["sh"], meta["np_nodes"]
    n_real, f_in = feat.shape
    kt = f_in // 128
    hd = W1.shape[1]
    heads, hidd = al1.shape if al1.ndim == 2 else (1, al1.shape[0])
    W1al = np.einsum("fhd,hd->fh", W1.reshape(f_in, heads, hidd), al1)
    W1ar = np.einsum("fhd,hd->fh", W1.reshape(f_in, heads, hidd), ar1)
    W1e = np.concatenate([W1, W1al, W1ar], axis=1)          # [f_in, we]
    we = W1e.shape[1]
    W1er = np.ascontiguousarray(
        W1e.reshape(kt, 128, we).transpose(1, 0, 2)).astype(BF16)
    b1rep = np.tile(b1.reshape(1, -1), (128, 1)).astype(np.float32)
    W2a = np.concatenate(
        [W2, W2 @ al2.reshape(-1, 1), W2 @ ar2.reshape(-1, 1)],
        axis=1).astype(BF16)
    b2rep = np.tile(b2.reshape(1, -1), (128, 1)).astype(np.float32)

    featP = np.zeros((np_nodes, f_in), dtype=np.float32)
    featP[:n_real] = feat
    fpi = featP[meta["row_node"]]                            # pi-ordered
    featT = np.ascontiguousarray(
        fpi.T.reshape(kt, 128, np_nodes).transpose(1, 0, 2)).astype(BF16)

    in_maps = []
    for c in range(NC):
        fo = featP[c * sh:(c + 1) * sh]
        featO = np.ascontiguousarray(
            fo.T.reshape(kt, 128, sh).transpose(1, 0, 2)).astype(BF16)
        in_maps.append({
            "featT": featT, "featO": featO,
            "W1e": W1er, "b1rep": b1rep,
            "W2a": W2a, "b2rep": b2rep,
            "gidx": _wrap16(meta["gidx"][c]),
            "slot": np.ascontiguousarray(
                meta["slot"][c].reshape(-1, 128).T).astype(BF16),
        })
    return in_maps


class Runner:
    """Builds the SPMD program once; exposes a repeatable timed executor."""

    def __init__(self, meta, f_in):
        self.meta = meta
        self.nc = build_program(meta, f_in, HID, HEADS, N_CLASSES)
        self._fn = None
        self.repeat = 1

    repeat = 1

    def _lower(self):
        import jax
        import numpy as _np
        from jax.sharding import Mesh, PartitionSpec
        from jax.experimental.shard_map import shard_map
        from concourse import mybir
        from concourse.bass2jax import _bass_exec_p, install_neuronx_cc_hook

        install_neuronx_cc_hook()
        nc = self.nc
        in_names, out_names, out_avals, zero_outs = [], [], [], []
        partition_name = (nc.partition_id_tensor.name
                          if nc.partition_id_tensor else None)
        for alloc in nc.m.functions[0].allocations:
            if not isinstance(alloc, mybir.MemoryLocationSet):
                continue
            name = alloc.memorylocations[0].name
            if alloc.kind == "ExternalInput":
                if name != partition_name:
                    in_names.append(name)
            elif alloc.kind == "ExternalOutput":
                shape = tuple(alloc.tensor_shape)
                dtype = mybir.dt.np(alloc.dtype)
                out_names.append(name)
                out_avals.append(jax.core.ShapedArray(shape, dtype))
                zero_outs.append(_np.zeros(shape, dtype))
        n_params = len(in_names)
        n_outs = len(out_avals)
        all_in_names = list(in_names) + list(out_names)
        if partition_name is not None:
            all_in_names.append(partition_name)

        repeat = self.repeat

        def _body(*args):
            ins = list(args[:n_params])
            zouts = list(args[n_params:])
            outs = None
            for _ in range(repeat):
                operands = ins + zouts
                if partition_name is not None:
                    from concourse.bass2jax import partition_id_tensor
                    operands.append(partition_id_tensor())
                outs = _bass_exec_p.bind(
                    *operands, out_avals=tuple(out_avals),
                    in_names=tuple(all_in_names), out_names=tuple(out_names),
                    lowering_input_output_aliases=(),
                    sim_require_finite=False, sim_require_nnan=False, nc=nc)
                zouts = list(outs)   # chain: serialize + reuse as out bufs
            return tuple(outs)

        devices = jax.devices()[:NC]
        mesh = Mesh(_np.asarray(devices), ("core",))
        in_specs = (PartitionSpec("core"),) * (n_params + n_outs)
        out_specs = (PartitionSpec("core"),) * n_outs
        self._fn = jax.jit(
            shard_map(_body, mesh=mesh, in_specs=in_specs,
                      out_specs=out_specs, check_rep=False),
            keep_unused=True)
        self._in_names = in_names
        self._out_names = out_names
        self._out_avals = out_avals
        self._zero_outs = zero_outs
        self._mesh = mesh
        self._in_specs = in_specs

    def prepare(self, in_maps):
        import jax
        import numpy as _np
        from jax.sharding import NamedSharding, PartitionSpec
        if self._fn is None:
            self._lower()
        concat_in = [
            _np.concatenate([in_maps[c][name] for c in range(NC)], axis=0)
            for name in self._in_names]
        concat_zeros = [
            _np.zeros((NC * z.shape[0], *z.shape[1:]), z.dtype)
            for z in self._zero_outs]
        shd = NamedSharding(self._mesh, PartitionSpec("core"))
        self._args = [jax.device_put(a, shd) for a in concat_in + concat_zeros]
        jax.block_until_ready(self._args)

    def run(self):
        import jax
        out = self._fn(*self._args)
        out = jax.block_until_ready(out)
        import numpy as _np
        res = _np.asarray(out[self._out_names.index("out")])
        sh = self._out_avals[self._out_names.index("out")].shape
        return res.reshape(NC, *sh).reshape(NC * sh[0], *sh[1:])


_RUNNER = None


def get_runner(feat, src, dst):
    global _RUNNER
    n, f_in = feat.shape
    meta = host_prep(np.asarray(src, np.int32), np.asarray(dst, np.int32),
                     n_nodes=n)
    _RUNNER = Runner(meta, f_in)
    return _RUNNER


def kernel(feat, src, dst, W1, al1, ar1, b1, W2, al2, ar2, b2):
    feat = np.asarray(feat, dtype=np.float32)
    src = np.asarray(src, dtype=np.int32)
    dst = np.asarray(dst, dtype=np.int32)
    args = [np.asarray(x, np.float32)
            for x in (W1, al1, ar1, b1, W2, al2, ar2, b2)]
    r = _RUNNER if _RUNNER is not None else get_runner(feat, src, dst)
    in_maps = make_in_maps(r.meta, feat, *args)
    r.prepare(in_maps)
    return r.run()[:feat.shape[0]]


kernel.last_exec_time_ns = None
